# revision 39
# baseline (speedup 1.0000x reference)
"""Chamfer distance (CDLoss) Trainium2 Bass kernel.

Full inputs: pcs1 [8, 8192, 3] f32, pcs2 [8, 8192, 3] f32.
Output: scalar f32 = mean(min-dist pcs1->pcs2) + mean(min-dist pcs2->pcs1).

Sharding: data-parallel over batch; core b handles cloud b. Each core
returns a [1,1] scalar loss for its cloud; host averages the 8 scalars.

Per-core algorithm:
  d[n,m] = |x_n|^2 + |y_m|^2 - 2 x_n.y_m is produced directly by the PE as
  an augmented inner product. fp32 matmuls stream at 1/4 rate on TRN2, so
  each f32 operand is split into three bf16 components (hi/mid/lo, capturing
  ~25 mantissa bits) and the six significant cross-product blocks are packed
  into one K=24 bf16 matmul (K rows are free on the PE; only streamed
  columns cost cycles). Distance error ~1e-6 absolute - f32 grade.

  Each [128 n, 2048 m] f32 PSUM distance tile is then:
    - copied to SBUF as bf16 by the Scalar engine (ACT, PSUM-adjacent),
    - folded by DVE in bf16 2x mode: a running elementwise min per column
      block (dist2 direction) and a pairwise min tree + reduce per n-tile
      row (dist1 direction).
  Partition-axis folds at the end go through PE transpose + DVE reduce, and
  the final cross-partition sum is a K=128 matmul against a ones vector.
"""

import sys
from contextlib import ExitStack

import numpy as np

if "/opt/trn_rl_repo" not in sys.path:
    sys.path.insert(0, "/opt/trn_rl_repo")

import concourse.bacc as bacc
import concourse.tile as tile
from concourse import bass_utils, mybir
from concourse.alu_op_type import AluOpType
from concourse.masks import make_identity

P = 128          # partitions
NPTS = 8192      # points per cloud (both clouds)
T = 64           # point tiles of 128
MSUP = 4         # m superblocks
MSB = 2048       # m superblock width (4 PSUM banks)
K = 24           # augmented contraction dim (6 cross blocks + 2x3 norm rows)
BIG = 1e30
F32 = mybir.dt.float32
BF16 = mybir.dt.bfloat16
B = 8            # batch / cores


def _split3(nc, pool, v, tag):
    """Split f32 tensor v into three bf16 components h+m+l ~ v (~25 bits)."""
    h = pool.tile(list(v.shape), BF16, tag=f"{tag}h")
    nc.vector.tensor_copy(out=h, in_=v)
    r1 = pool.tile(list(v.shape), F32, tag=f"{tag}r1")
    nc.vector.tensor_tensor(out=r1, in0=v, in1=h, op=AluOpType.subtract)
    m = pool.tile(list(v.shape), BF16, tag=f"{tag}m")
    nc.vector.tensor_copy(out=m, in_=r1)
    r2 = pool.tile(list(v.shape), F32, tag=f"{tag}r2")
    nc.vector.tensor_tensor(out=r2, in0=r1, in1=m, op=AluOpType.subtract)
    l = pool.tile(list(v.shape), BF16, tag=f"{tag}l")
    nc.vector.tensor_copy(out=l, in_=r2)
    return h, m, l


def _build_aug(nc, work, pts, x_side, tag):
    """pts [P,T,3] f32 -> bf16 augmented [P,T,24].

    x-side rows: [ah ah am ah al am]*3, ch cm cl, 1 1 1   (a = -2x, c=|x|^2)
    y-side rows: [yh ym yh yl yh ym]*3, 1 1 1, gh gm gl   (g = |y|^2)
    Pairing gives -2x.y (hh+hm+mh+hl+lh+mm blocks) + |x|^2 + |y|^2.
    """
    prod = work.tile([P, T, 3], F32, tag=f"{tag}prod")
    nc.vector.tensor_tensor(out=prod, in0=pts, in1=pts, op=AluOpType.mult)
    v = work.tile([P, T, 4], F32, tag=f"{tag}v")
    if x_side:
        nc.vector.tensor_scalar_mul(v[:, :, 0:3], pts, -2.0)
    else:
        nc.vector.tensor_copy(out=v[:, :, 0:3], in_=pts)
    nc.vector.tensor_reduce(
        out=v[:, :, 3:4], in_=prod, axis=mybir.AxisListType.X, op=AluOpType.add
    )
    h, m, l = _split3(nc, work, v, tag)

    aug = work.tile([P, T, K], BF16, tag=f"{tag}aug")
    if x_side:
        blocks = [h, h, m, h, l, m]
    else:
        blocks = [h, m, h, l, h, m]
    for bi, blk in enumerate(blocks):
        nc.vector.tensor_copy(
            out=aug[:, :, 3 * bi:3 * bi + 3], in_=blk[:, :, 0:3]
        )
    if x_side:
        nc.vector.tensor_copy(out=aug[:, :, 18:19], in_=h[:, :, 3:4])
        nc.vector.tensor_copy(out=aug[:, :, 19:20], in_=m[:, :, 3:4])
        nc.vector.tensor_copy(out=aug[:, :, 20:21], in_=l[:, :, 3:4])
        nc.vector.memset(aug[:, :, 21:24], 1.0)
    else:
        nc.vector.memset(aug[:, :, 18:21], 1.0)
        nc.vector.tensor_copy(out=aug[:, :, 21:22], in_=h[:, :, 3:4])
        nc.vector.tensor_copy(out=aug[:, :, 22:23], in_=m[:, :, 3:4])
        nc.vector.tensor_copy(out=aug[:, :, 23:24], in_=l[:, :, 3:4])
    return aug


def build_program():
    nc = bacc.Bacc("TRN2", target_bir_lowering=False, debug=False, num_devices=B)
    pcs1 = nc.dram_tensor("pcs1", [NPTS, 3], F32, kind="ExternalInput").ap()
    pcs2 = nc.dram_tensor("pcs2", [NPTS, 3], F32, kind="ExternalInput").ap()
    out = nc.dram_tensor("out", [1, 1], F32, kind="ExternalOutput").ap()

    with ExitStack() as ctx:
        tc = ctx.enter_context(tile.TileContext(nc))
        consts = ctx.enter_context(tc.tile_pool(name="consts", bufs=1))
        work = ctx.enter_context(tc.tile_pool(name="work", bufs=1))
        dcp = ctx.enter_context(tc.tile_pool(name="dcp", bufs=12))
        tree = ctx.enter_context(tc.tile_pool(name="tree", bufs=3))
        ps_pool = ctx.enter_context(tc.tile_pool(name="ps", bufs=2, space="PSUM"))

        # ---- load points: n = p*64 + t so each partition reads contiguous ----
        X = consts.tile([P, T, 3], F32)
        nc.sync.dma_start(out=X, in_=pcs1.rearrange("(p t) d -> p t d", p=P))
        Y = consts.tile([P, T, 3], F32)
        nc.sync.dma_start(out=Y, in_=pcs2.rearrange("(p t) d -> p t d", p=P))
        # make_identity runs on gpsimd; keep PE deps funneled through DVE.
        ident_g = consts.tile([P, P], F32)
        make_identity(nc, ident_g)
        identb = consts.tile([P, P], BF16)
        nc.vector.tensor_copy(out=identb, in_=ident_g)

        # ---- bf16 split + augmented 24-vectors (YA first, XA last) ----
        YA = _build_aug(nc, work, Y, x_side=False, tag="y")
        XA = _build_aug(nc, work, X, x_side=True, tag="x")

        # ---- transpose phase helpers: [128, 24] -> [24, 128] K-major ----
        WX = consts.tile([K, NPTS], BF16)
        WY = consts.tile([K, NPTS], BF16)

        def emit_w_block(src, dst, blk):
            pst = ps_pool.tile([P, MSB], BF16, tag="ps")
            for r in range(16):
                t = blk * 16 + r
                nc.tensor.transpose(
                    pst[0:K, r * P:(r + 1) * P], src[:, t, :], identb
                )
            nc.scalar.copy(
                out=dst[:, blk * MSB:(blk + 1) * MSB], in_=pst[0:K, :]
            )

        for blk in range(T // 16):
            emit_w_block(YA, WY, blk)
        for blk in range(T // 16):
            emit_w_block(XA, WX, blk)

        # ---- main loop: distance tiles + min extraction ----
        col_acc = consts.tile([P, MSUP, MSB], BF16)
        nc.vector.memset(col_acc, BIG)
        row_parts = consts.tile([P, T, MSUP], F32)

        for i in range(T):
            lhsT = WX[:, i * P:(i + 1) * P]
            for j in range(MSUP):
                ps = ps_pool.tile([P, MSB], F32, tag="ps")
                for k in range(MSB // 512):
                    nc.tensor.matmul(
                        ps[:, k * 512:(k + 1) * 512],
                        lhsT=lhsT,
                        rhs=WY[:, j * MSB + k * 512: j * MSB + (k + 1) * 512],
                        start=True,
                        stop=True,
                    )
                dcopy = dcp.tile([P, MSB], BF16, tag="dcopy")
                # a few groups take the fused DVE path (PSUM copy + exact
                # f32 row-min in one 1x op) to offload the busier ACT engine
                dve_copies = i % 7 == 3 and j == 3
                if dve_copies:
                    nc.vector.tensor_scalar(
                        out=dcopy,
                        in0=ps,
                        scalar1=BIG,
                        scalar2=None,
                        op0=AluOpType.min,
                        op1=AluOpType.min,
                        accum_out=row_parts[:, i, j:j + 1],
                    )
                else:
                    nc.scalar.copy(out=dcopy, in_=ps)
                # bf16 2x-mode column-direction fold
                nc.vector.tensor_tensor(
                    out=col_acc[:, j, :],
                    in0=col_acc[:, j, :],
                    in1=dcopy,
                    op=AluOpType.min,
                )
                if not dve_copies:
                    # row-direction min of this block: single-src bf16 SBUF
                    # tensor_scalar runs in DVE 4x mode; accum_out carries
                    # the reduce-min (out itself is a throwaway)
                    junk = tree.tile([P, MSB], BF16, tag="junk")
                    nc.vector.tensor_scalar(
                        out=junk,
                        in0=dcopy,
                        scalar1=BIG,
                        scalar2=None,
                        op0=AluOpType.min,
                        op1=AluOpType.min,
                        accum_out=row_parts[:, i, j:j + 1],
                    )

        # ---- finalize: dist1 ----
        dist1 = work.tile([P, T], F32, tag="dist1")
        nc.vector.tensor_reduce(
            out=dist1, in_=row_parts, axis=mybir.AxisListType.X, op=AluOpType.min
        )
        s1 = work.tile([P, 1], F32, tag="s1")
        nc.vector.tensor_reduce(
            out=s1, in_=dist1, axis=mybir.AxisListType.X, op=AluOpType.add
        )

        # ---- finalize: dist2 (fold 128 partitions via PE transpose) ----
        dist2 = work.tile([P, T], F32, tag="dist2")
        for blk in range(MSUP):
            pst = ps_pool.tile([P, MSB], BF16, tag="ps")
            for r in range(16):
                nc.tensor.transpose(
                    pst[:, r * P:(r + 1) * P],
                    col_acc[:, blk, r * P:(r + 1) * P],
                    identb,
                )
            nc.vector.tensor_reduce(
                out=dist2[:, blk * 16:(blk + 1) * 16],
                in_=pst.rearrange("p (r c) -> p r c", c=P),
                axis=mybir.AxisListType.X,
                op=AluOpType.min,
            )
        s2 = work.tile([P, 1], F32, tag="s2")
        nc.vector.tensor_reduce(
            out=s2, in_=dist2, axis=mybir.AxisListType.X, op=AluOpType.add
        )

        # ---- combine + cross-partition sum via ones-matmul ----
        comb = work.tile([P, 1], F32, tag="comb")
        nc.vector.tensor_tensor(out=comb, in0=s1, in1=s2, op=AluOpType.add)
        scl = work.tile([P, 1], F32, tag="scl")
        nc.vector.tensor_scalar_mul(scl, comb, 1.0 / NPTS)
        ones = consts.tile([P, 1], F32)
        nc.vector.memset(ones, 1.0)
        psc = ps_pool.tile([P, MSB], F32, tag="ps")
        nc.tensor.matmul(psc[0:1, 0:1], lhsT=scl, rhs=ones, start=True, stop=True)
        outsb = work.tile([1, 1], F32, tag="outsb")
        nc.vector.tensor_copy(out=outsb, in_=psc[0:1, 0:1])
        nc.sync.dma_start(out=out, in_=outsb)

    nc.compile()
    return nc


_NC_CACHE = None


def _get_nc():
    global _NC_CACHE
    if _NC_CACHE is None:
        _NC_CACHE = build_program()
    return _NC_CACHE


def run(pcs1, pcs2, trace=False):
    nc = _get_nc()
    pcs1 = np.ascontiguousarray(np.asarray(pcs1, dtype=np.float32))
    pcs2 = np.ascontiguousarray(np.asarray(pcs2, dtype=np.float32))
    assert pcs1.shape == (B, NPTS, 3) and pcs2.shape == (B, NPTS, 3)
    in_maps = [{"pcs1": pcs1[b], "pcs2": pcs2[b]} for b in range(B)]
    res = bass_utils.run_bass_kernel_spmd(
        nc, in_maps, core_ids=list(range(B)), trace=trace
    )
    vals = np.array(
        [res.results[b]["out"][0, 0] for b in range(B)], dtype=np.float64
    )
    return np.float32(vals.mean()), res


def kernel(pcs1, pcs2):
    val, _ = run(pcs1, pcs2, trace=False)
    return val


# revision 40
# speedup vs baseline: 1.0039x; 1.0039x over previous
"""Chamfer distance (CDLoss) Trainium2 Bass kernel.

Full inputs: pcs1 [8, 8192, 3] f32, pcs2 [8, 8192, 3] f32.
Output: scalar f32 = mean(min-dist pcs1->pcs2) + mean(min-dist pcs2->pcs1).

Sharding: data-parallel over batch; core b handles cloud b. Each core
returns a [1,1] scalar loss for its cloud; host averages the 8 scalars.

Per-core algorithm:
  d[n,m] = |x_n|^2 + |y_m|^2 - 2 x_n.y_m is produced directly by the PE as
  an augmented inner product. fp32 matmuls stream at 1/4 rate on TRN2, so
  each f32 operand is split into three bf16 components (hi/mid/lo, capturing
  ~25 mantissa bits) and the six significant cross-product blocks are packed
  into one K=24 bf16 matmul (K rows are free on the PE; only streamed
  columns cost cycles). Distance error ~1e-6 absolute - f32 grade.

  Each [128 n, 2048 m] f32 PSUM distance tile is then:
    - copied to SBUF as bf16 by the Scalar engine (ACT, PSUM-adjacent),
    - folded by DVE in bf16 2x mode: a running elementwise min per column
      block (dist2 direction) and a pairwise min tree + reduce per n-tile
      row (dist1 direction).
  Partition-axis folds at the end go through PE transpose + DVE reduce, and
  the final cross-partition sum is a K=128 matmul against a ones vector.
"""

import sys
from contextlib import ExitStack

import numpy as np

if "/opt/trn_rl_repo" not in sys.path:
    sys.path.insert(0, "/opt/trn_rl_repo")

import concourse.bacc as bacc
import concourse.tile as tile
from concourse import bass_utils, mybir
from concourse.alu_op_type import AluOpType
from concourse.masks import make_identity

P = 128          # partitions
NPTS = 8192      # points per cloud (both clouds)
T = 64           # point tiles of 128
MSUP = 4         # m superblocks
MSB = 2048       # m superblock width (4 PSUM banks)
K = 24           # augmented contraction dim (6 cross blocks + 2x3 norm rows)
BIG = 1e30
F32 = mybir.dt.float32
BF16 = mybir.dt.bfloat16
B = 8            # batch / cores


def _split3(nc, pool, v, tag):
    """Split f32 tensor v into three bf16 components h+m+l ~ v (~25 bits)."""
    h = pool.tile(list(v.shape), BF16, tag=f"{tag}h")
    nc.vector.tensor_copy(out=h, in_=v)
    r1 = pool.tile(list(v.shape), F32, tag=f"{tag}r1")
    nc.vector.tensor_tensor(out=r1, in0=v, in1=h, op=AluOpType.subtract)
    m = pool.tile(list(v.shape), BF16, tag=f"{tag}m")
    nc.vector.tensor_copy(out=m, in_=r1)
    r2 = pool.tile(list(v.shape), F32, tag=f"{tag}r2")
    nc.vector.tensor_tensor(out=r2, in0=r1, in1=m, op=AluOpType.subtract)
    l = pool.tile(list(v.shape), BF16, tag=f"{tag}l")
    nc.vector.tensor_copy(out=l, in_=r2)
    return h, m, l


def _build_aug(nc, work, pts, x_side, tag):
    """pts [P,T,3] f32 -> bf16 augmented [P,T,24].

    x-side rows: [ah ah am ah al am]*3, ch cm cl, 1 1 1   (a = -2x, c=|x|^2)
    y-side rows: [yh ym yh yl yh ym]*3, 1 1 1, gh gm gl   (g = |y|^2)
    Pairing gives -2x.y (hh+hm+mh+hl+lh+mm blocks) + |x|^2 + |y|^2.
    """
    prod = work.tile([P, T, 3], F32, tag=f"{tag}prod")
    nc.vector.tensor_tensor(out=prod, in0=pts, in1=pts, op=AluOpType.mult)
    v = work.tile([P, T, 4], F32, tag=f"{tag}v")
    if x_side:
        nc.vector.tensor_scalar_mul(v[:, :, 0:3], pts, -2.0)
    else:
        nc.vector.tensor_copy(out=v[:, :, 0:3], in_=pts)
    nc.vector.tensor_reduce(
        out=v[:, :, 3:4], in_=prod, axis=mybir.AxisListType.X, op=AluOpType.add
    )
    h, m, l = _split3(nc, work, v, tag)

    aug = work.tile([P, T, K], BF16, tag=f"{tag}aug")
    if x_side:
        blocks = [h, h, m, h, l, m]
    else:
        blocks = [h, m, h, l, h, m]
    for bi, blk in enumerate(blocks):
        nc.vector.tensor_copy(
            out=aug[:, :, 3 * bi:3 * bi + 3], in_=blk[:, :, 0:3]
        )
    if x_side:
        nc.vector.tensor_copy(out=aug[:, :, 18:19], in_=h[:, :, 3:4])
        nc.vector.tensor_copy(out=aug[:, :, 19:20], in_=m[:, :, 3:4])
        nc.vector.tensor_copy(out=aug[:, :, 20:21], in_=l[:, :, 3:4])
        nc.vector.memset(aug[:, :, 21:24], 1.0)
    else:
        nc.vector.memset(aug[:, :, 18:21], 1.0)
        nc.vector.tensor_copy(out=aug[:, :, 21:22], in_=h[:, :, 3:4])
        nc.vector.tensor_copy(out=aug[:, :, 22:23], in_=m[:, :, 3:4])
        nc.vector.tensor_copy(out=aug[:, :, 23:24], in_=l[:, :, 3:4])
    return aug


def build_program():
    nc = bacc.Bacc("TRN2", target_bir_lowering=False, debug=False, num_devices=B)
    pcs1 = nc.dram_tensor("pcs1", [NPTS, 3], F32, kind="ExternalInput").ap()
    pcs2 = nc.dram_tensor("pcs2", [NPTS, 3], F32, kind="ExternalInput").ap()
    out = nc.dram_tensor("out", [1, 1], F32, kind="ExternalOutput").ap()

    with ExitStack() as ctx:
        tc = ctx.enter_context(tile.TileContext(nc))
        consts = ctx.enter_context(tc.tile_pool(name="consts", bufs=1))
        work = ctx.enter_context(tc.tile_pool(name="work", bufs=1))
        dcp = ctx.enter_context(tc.tile_pool(name="dcp", bufs=12))
        tree = ctx.enter_context(tc.tile_pool(name="tree", bufs=3))
        ps_pool = ctx.enter_context(tc.tile_pool(name="ps", bufs=2, space="PSUM"))

        # ---- load points: n = p*64 + t so each partition reads contiguous ----
        X = consts.tile([P, T, 3], F32)
        nc.sync.dma_start(out=X, in_=pcs1.rearrange("(p t) d -> p t d", p=P))
        Y = consts.tile([P, T, 3], F32)
        nc.sync.dma_start(out=Y, in_=pcs2.rearrange("(p t) d -> p t d", p=P))
        # make_identity runs on gpsimd; keep PE deps funneled through DVE.
        ident_g = consts.tile([P, P], F32)
        make_identity(nc, ident_g)
        identb = consts.tile([P, P], BF16)
        nc.vector.tensor_copy(out=identb, in_=ident_g)

        # ---- bf16 split + augmented 24-vectors (YA first, XA last) ----
        YA = _build_aug(nc, work, Y, x_side=False, tag="y")
        XA = _build_aug(nc, work, X, x_side=True, tag="x")

        # ---- transpose phase helpers: [128, 24] -> [24, 128] K-major ----
        WX = consts.tile([K, NPTS], BF16)
        WY = consts.tile([K, NPTS], BF16)

        def emit_w_block(src, dst, blk):
            pst = ps_pool.tile([P, MSB], BF16, tag="ps")
            for r in range(16):
                t = blk * 16 + r
                nc.tensor.transpose(
                    pst[0:K, r * P:(r + 1) * P], src[:, t, :], identb
                )
            nc.scalar.copy(
                out=dst[:, blk * MSB:(blk + 1) * MSB], in_=pst[0:K, :]
            )

        for blk in range(T // 16):
            emit_w_block(YA, WY, blk)
        for blk in range(T // 16):
            emit_w_block(XA, WX, blk)

        # ---- main loop: distance tiles + min extraction ----
        col_acc = consts.tile([P, MSUP, MSB], BF16)
        nc.vector.memset(col_acc, BIG)
        row_parts = consts.tile([P, T, MSUP], F32)

        for i in range(T):
            lhsT = WX[:, i * P:(i + 1) * P]
            for j in range(MSUP):
                ps = ps_pool.tile([P, MSB], F32, tag="ps")
                for k in range(MSB // 512):
                    nc.tensor.matmul(
                        ps[:, k * 512:(k + 1) * 512],
                        lhsT=lhsT,
                        rhs=WY[:, j * MSB + k * 512: j * MSB + (k + 1) * 512],
                        start=True,
                        stop=True,
                    )
                dcopy = dcp.tile([P, MSB], BF16, tag="dcopy")
                # a few groups take the fused DVE path (PSUM copy + exact
                # f32 row-min in one 1x op) to offload the busier ACT engine
                dve_copies = i % 7 == 3 and j == 3
                if dve_copies:
                    nc.vector.tensor_scalar(
                        out=dcopy,
                        in0=ps,
                        scalar1=BIG,
                        scalar2=None,
                        op0=AluOpType.min,
                        op1=AluOpType.min,
                        accum_out=row_parts[:, i, j:j + 1],
                    )
                else:
                    nc.scalar.copy(out=dcopy, in_=ps)
                # bf16 2x-mode column-direction fold
                nc.vector.tensor_tensor(
                    out=col_acc[:, j, :],
                    in0=col_acc[:, j, :],
                    in1=dcopy,
                    op=AluOpType.min,
                )
                if not dve_copies:
                    # row-direction min of this block: single-src bf16 SBUF
                    # tensor_scalar runs in DVE 4x mode; accum_out carries
                    # the reduce-min (out itself is a throwaway)
                    junk = tree.tile([P, MSB], BF16, tag="junk")
                    nc.vector.tensor_scalar(
                        out=junk,
                        in0=dcopy,
                        scalar1=BIG,
                        scalar2=None,
                        op0=AluOpType.min,
                        op1=AluOpType.min,
                        accum_out=row_parts[:, i, j:j + 1],
                    )

        # ---- finalize: dist1 ----
        dist1 = work.tile([P, T], F32, tag="dist1")
        nc.vector.tensor_reduce(
            out=dist1, in_=row_parts, axis=mybir.AxisListType.X, op=AluOpType.min
        )
        s1 = work.tile([P, 1], F32, tag="s1")
        nc.vector.tensor_reduce(
            out=s1, in_=dist1, axis=mybir.AxisListType.X, op=AluOpType.add
        )

        # ---- finalize: dist2 (fold 128 partitions via PE transpose) ----
        dist2 = work.tile([P, T], F32, tag="dist2")
        for blk in range(MSUP):
            pst = ps_pool.tile([P, MSB], BF16, tag="ps")
            for r in range(16):
                nc.tensor.transpose(
                    pst[:, r * P:(r + 1) * P],
                    col_acc[:, blk, r * P:(r + 1) * P],
                    identb,
                )
            dc2 = dcp.tile([P, MSB], BF16, tag="dcopy")
            nc.scalar.copy(out=dc2, in_=pst)
            junk2 = tree.tile([P, MSB], BF16, tag="junk")
            for r in range(16):
                nc.vector.tensor_scalar(
                    out=junk2[:, r * P:(r + 1) * P],
                    in0=dc2[:, r * P:(r + 1) * P],
                    scalar1=BIG,
                    scalar2=None,
                    op0=AluOpType.min,
                    op1=AluOpType.min,
                    accum_out=dist2[:, blk * 16 + r:blk * 16 + r + 1],
                )
        s2 = work.tile([P, 1], F32, tag="s2")
        nc.vector.tensor_reduce(
            out=s2, in_=dist2, axis=mybir.AxisListType.X, op=AluOpType.add
        )

        # ---- combine + cross-partition sum via ones-matmul ----
        comb = work.tile([P, 1], F32, tag="comb")
        nc.vector.tensor_tensor(out=comb, in0=s1, in1=s2, op=AluOpType.add)
        scl = work.tile([P, 1], F32, tag="scl")
        nc.vector.tensor_scalar_mul(scl, comb, 1.0 / NPTS)
        ones = consts.tile([P, 1], F32)
        nc.vector.memset(ones, 1.0)
        psc = ps_pool.tile([P, MSB], F32, tag="ps")
        nc.tensor.matmul(psc[0:1, 0:1], lhsT=scl, rhs=ones, start=True, stop=True)
        outsb = work.tile([1, 1], F32, tag="outsb")
        nc.vector.tensor_copy(out=outsb, in_=psc[0:1, 0:1])
        nc.sync.dma_start(out=out, in_=outsb)

    nc.compile()
    return nc


_NC_CACHE = None


def _get_nc():
    global _NC_CACHE
    if _NC_CACHE is None:
        _NC_CACHE = build_program()
    return _NC_CACHE


def run(pcs1, pcs2, trace=False):
    nc = _get_nc()
    pcs1 = np.ascontiguousarray(np.asarray(pcs1, dtype=np.float32))
    pcs2 = np.ascontiguousarray(np.asarray(pcs2, dtype=np.float32))
    assert pcs1.shape == (B, NPTS, 3) and pcs2.shape == (B, NPTS, 3)
    in_maps = [{"pcs1": pcs1[b], "pcs2": pcs2[b]} for b in range(B)]
    res = bass_utils.run_bass_kernel_spmd(
        nc, in_maps, core_ids=list(range(B)), trace=trace
    )
    vals = np.array(
        [res.results[b]["out"][0, 0] for b in range(B)], dtype=np.float64
    )
    return np.float32(vals.mean()), res


def kernel(pcs1, pcs2):
    val, _ = run(pcs1, pcs2, trace=False)
    return val
